# revision 14
# baseline (speedup 1.0000x reference)
"""Trainium2 Bass kernel for BlankEmbedding (embedding lookup + blank shift-accumulate).

Reference semantics:
    out = emb[x]                                    # [B, S, D] gather
    preblank[p] = (x[p+1]==BLANK) & (x[p]!=BLANK)   (per row; zero-padded shifts)
    out[p+k] += preblank[p] * emb[x[p]]  for k in 1..3

Strategy: data-parallel over the 16384 flattened tokens, 2048 per core.
The device does a pure int8 row gather + passthrough store; the host
dequantizes (global absmax/127 scale, ~7.8e-3 rel err vs the 2e-2 budget)
and places rows while unsharding. Fixups (P(blank)=1/50257) are recomputed
on-device in int16 and dropped in by the host (placement only).

Measured HW facts this design is built on (micro-benchmarked on trn2):
- All SWDGE descgen runs on the Pool Q7s at ~8.5ns/row engine-blocking,
  BUT InstDMAGatherAnt instructions on queues 1-3 dispatch in ~70ns and
  their descgen runs on a background worker at ~2.5ns/row. Queue 0 and
  the first SWDGE instruction of the program stay engine-synchronous.
- One dma_gather must carry <= 1024 indices (1280 wedges the Q7).
- dma_gather needs the mlp ucode library: ~9us DMA load, engine-blocking,
  started by an explicit load_library as the first gpsimd instruction.
- Indices are int16, so the 50258-row table ships as two halves with a
  zero row each: emb8a[0]=0, emb8a[1+r]=row r (r<32767); emb8b[r-32767]=
  row r (r>=32767), emb8b[17491]=0. Every vocab value maps into each
  half (zero row when absent), which also makes the fixup adds
  select-free: emb[v] = emb8a[map_a(v)] + emb8b[map_b(v)].
- Gather list position j lands at tile[j%128, j//128]; idx tiles are
  int16 [128, n/16] with idx j at [j%16, j//16], replicated 8x across
  partitions. Per-core A/B counts vary; capacities ka/kb are maxed over
  cores (SPMD: one program), padded with index 0.
- Fixup gather k/16+k/32+k slots hold xt/s1/s2 of fix slot k; the two
  half-gathers are summed (int16), then the s1/s2 partition groups are
  realigned with two tiny SBUF-to-SBUF DMAs and added.
"""

import numpy as np

VOCAB = 50257
DIM = 1024
BLANK = 100
N_BLANKS = 3
B, S = 4, 4096
N_CORES = 8
TOK = B * S                  # 16384 flattened tokens
TPC = TOK // N_CORES         # 2048 tokens per core
P = 128                      # SBUF partitions
ASPLIT = 32767               # values < ASPLIT live in half A
NB_ROWS = VOCAB - ASPLIT + 1  # 17491: B rows + trailing zero row
BZERO = NB_ROWS - 1          # emb8b zero-row index
KFIX = 16
CHUNK = 1024                 # HW limit per dma_gather instruction

_CACHE = {}


def _chunks(total):
    """Split a 128-multiple count into <=CHUNK 128-multiple chunks."""
    out = []
    left = total
    while left > 0:
        n = min(CHUNK, left)
        out.append(n)
        left -= n
    return out


def _build_nc(ka, kb):
    from concourse import bacc, mybir, tile, library_config

    wa, wb = ka // 16, kb // 16
    ca, cb = ka // P, kb // P

    nc = bacc.Bacc(
        "TRN2", target_bir_lowering=False, debug=False, num_devices=1,
        num_swdge_queues=4,
    )
    i8 = mybir.dt.int8
    i16 = mybir.dt.int16

    # idx words: [fxA(8) | fxB(8) | A(wa) | B(wb)]
    W = 16 + wa + wb
    ix_dram = nc.dram_tensor("ix", [P, W], i16, kind="ExternalInput")
    emb8a = nc.dram_tensor("emb8a", [ASPLIT + 1, DIM], i8, kind="ExternalInput")
    emb8b = nc.dram_tensor("emb8b", [NB_ROWS, DIM], i8, kind="ExternalInput")
    out = nc.dram_tensor("out", [P, (ca + cb) * DIM], i8, kind="ExternalOutput")
    fixout = nc.dram_tensor("fixout", [KFIX, DIM], i16, kind="ExternalOutput")

    with tile.TileContext(nc) as tc:
        with tc.tile_pool(name="sbuf", bufs=1) as pool:
            ixt = pool.tile([P, W], i16)
            nc.sync.dma_start(out=ixt[:], in_=ix_dram[:])

            big = pool.tile([P, (ca + cb) * DIM], i8)
            big3 = big[:].rearrange("p (c d) -> p c d", c=ca + cb, d=DIM)
            fxa = pool.tile([P, DIM], i8)
            fxb = pool.tile([P, DIM], i8)
            fxa3 = fxa[:].rearrange("p (c d) -> p c d", c=1, d=DIM)
            fxb3 = fxb[:].rearrange("p (c d) -> p c d", c=1, d=DIM)

            nc.gpsimd.load_library(library_config.mlp)
            # The first SWDGE instruction always engine-syncs: sacrifice the
            # small fxA (128 idx; slots 48+ hit the zero rows). The main
            # chunks then dispatch async (~70ns) onto the background descgen
            # worker; fxB goes last and runs on whichever stream is free.
            nc.gpsimd.dma_gather(fxa3[:, :, :], emb8a[:], ixt[:, 0:8],
                                 P, P, DIM, elem_step=DIM, queue_num=1)

            # main chunks, largest first, with word/col offsets precomputed
            chunks = []
            col = 0
            word = 16
            for src, total in ((emb8a, ka), (emb8b, kb)):
                for n in _chunks(total):
                    chunks.append((src, n, col, word))
                    col += n // P
                    word += n // 16
            chunks.sort(key=lambda t: -t[1])
            for i, (src, n, c0, w0) in enumerate(chunks):
                nc.gpsimd.dma_gather(
                    big3[:, c0 : c0 + n // P, :], src[:],
                    ixt[:, w0 : w0 + n // 16],
                    n, n, DIM, elem_step=DIM, queue_num=1 + i % 3,
                )
            nc.gpsimd.dma_gather(fxb3[:, :, :], emb8b[:], ixt[:, 8:16],
                                 P, P, DIM, elem_step=DIM, queue_num=2)
            for src, n, c0, w0 in chunks:
                nc.sync.dma_start(
                    out=out[:, c0 * DIM : (c0 + n // P) * DIM],
                    in_=big[:, c0 * DIM : (c0 + n // P) * DIM],
                )

            # fixup: wsum = widen(fxA) + widen(fxB) holds emb[xt_k] at
            # partition k, emb[s1_k] at 32+k, emb[s2_k] at 64+k (group
            # bases on 32-partition boundaries: DMA start-partition rule)
            wa16 = pool.tile([P, DIM], i16)
            wb16 = pool.tile([P, DIM], i16)
            for src_t, dst_t in ((fxa, wa16), (fxb, wb16)):
                nc.vector.tensor_scalar(
                    out=dst_t[:80, :], in0=src_t[:80, :],
                    scalar1=1.0, scalar2=None, op0=mybir.AluOpType.mult,
                )
            nc.vector.tensor_tensor(
                out=wa16[:80, :], in0=wa16[:80, :],
                in1=wb16[:80, :], op=mybir.AluOpType.add,
            )
            # realign s1/s2 groups onto partitions 0..15 and accumulate
            g1 = pool.tile([P, DIM], i16)
            g2 = pool.tile([P, DIM], i16)
            nc.scalar.dma_start(out=g1[0:KFIX, :], in_=wa16[32 : 32 + KFIX, :])
            nc.scalar.dma_start(out=g2[0:KFIX, :], in_=wa16[64 : 64 + KFIX, :])
            nc.vector.tensor_tensor(
                out=g1[0:KFIX, :], in0=g1[0:KFIX, :],
                in1=g2[0:KFIX, :], op=mybir.AluOpType.add,
            )
            nc.vector.tensor_tensor(
                out=wa16[0:KFIX, :], in0=wa16[0:KFIX, :],
                in1=g1[0:KFIX, :], op=mybir.AluOpType.add,
            )
            nc.scalar.dma_start(out=fixout[:], in_=wa16[:KFIX, :])

    nc.compile()
    return nc


def get_nc(ka, kb):
    key = (ka, kb)
    if key not in _CACHE:
        _CACHE[key] = _build_nc(ka, kb)
    return _CACHE[key]


def _corrections(x2):
    """Exact reference semantics: list of (global_target_row, src_token)."""
    is_blank = x2 == BLANK
    prev = np.zeros_like(is_blank)
    prev[:, 1:] = is_blank[:, :-1]
    first_blank = is_blank & ~prev
    out = []
    for b, f in np.argwhere(first_blank):
        if f == 0:
            continue  # run at row start: reference shifts in zeros
        p = f - 1
        src_tok = int(x2[b, p])
        for k in range(1, N_BLANKS + 1):
            s = p + k
            if s >= S:
                break
            out.append((b * S + s, src_tok))
    return out


def _round_up(n, m):
    return (n + m - 1) // m * m


def _idx_block(vals, cap):
    """int16 idx layout: idx j at [j%16, j//16], replicated to 128 rows."""
    padded = np.zeros(cap, dtype=np.int16)
    padded[: len(vals)] = vals
    block = padded.reshape(cap // 16, 16).T  # [16, cap//16]
    return np.tile(block, (P // 16, 1))      # [128, cap//16]


def _map_a(v):
    """Half-A local index for value v (zero row when v is in half B)."""
    v = np.asarray(v)
    return np.where(v < ASPLIT, v + 1, 0).astype(np.int16)


def _map_b(v):
    v = np.asarray(v)
    return np.where(v >= ASPLIT, v - ASPLIT, BZERO).astype(np.int16)


def shard_inputs(x, emb_table):
    """Returns (in_maps, perms, fix_targets, ka, kb, scale)."""
    x2 = np.asarray(x).astype(np.int64).reshape(B, S)
    flat = x2.reshape(-1).astype(np.int32)
    emb_f = np.asarray(emb_table, dtype=np.float32)
    scale = float(np.abs(emb_f).max()) / 127.0
    emb_i8 = np.clip(np.rint(emb_f / scale), -127, 127).astype(np.int8)
    zrow = np.zeros((1, DIM), dtype=np.int8)
    emb8a = np.ascontiguousarray(np.vstack([zrow, emb_i8[:ASPLIT]]))
    emb8b = np.ascontiguousarray(np.vstack([emb_i8[ASPLIT:], zrow]))

    per_tgt = {}
    for tgt, src in _corrections(x2):
        per_tgt.setdefault(tgt, []).append(src)
    assert all(len(v) <= 2 for v in per_tgt.values()), per_tgt

    orders = []
    for c in range(N_CORES):
        t = flat[c * TPC : (c + 1) * TPC]
        in_a = t < ASPLIT
        orders.append((np.nonzero(in_a)[0], np.nonzero(~in_a)[0]))
    ka = max(_round_up(len(oa), P) for oa, _ in orders)
    kb = max(_round_up(len(ob), P) for _, ob in orders)

    in_maps = []
    perms = []
    fix_targets = []
    for c in range(N_CORES):
        base = c * TPC
        t = flat[base : base + TPC]
        oa, ob = orders[c]

        # fixup idx groups: slot k -> xt at k, s1 at 32+k, s2 at 64+k
        fvals = np.full(P, -1, dtype=np.int64)  # -1 -> zero rows
        mine = {t_: v for t_, v in per_tgt.items() if base <= t_ < base + TPC}
        assert len(mine) <= KFIX, "fixup slot overflow"
        targets = {}
        for slot, (tgt, srcs) in enumerate(mine.items()):
            fvals[slot] = flat[tgt]
            fvals[32 + slot] = srcs[0]
            if len(srcs) > 1:
                fvals[64 + slot] = srcs[1]
            targets[slot] = tgt - base
        fix_targets.append(targets)
        fxa = np.where(fvals >= 0, _map_a(np.maximum(fvals, 0)), 0)
        fxb = np.where(fvals >= 0, _map_b(np.maximum(fvals, 0)), BZERO)

        ix = np.concatenate(
            [
                _idx_block(fxa.astype(np.int16), P),
                _idx_block(fxb.astype(np.int16), P),
                _idx_block(_map_a(t[oa]), ka),
                _idx_block(_map_b(t[ob]), kb),
            ],
            axis=1,
        )
        perm = np.empty(TPC, dtype=np.int64)
        perm[oa] = np.arange(len(oa))
        perm[ob] = ka + np.arange(len(ob))
        perms.append(perm)
        in_maps.append({"ix": ix, "emb8a": emb8a, "emb8b": emb8b})
    return in_maps, perms, fix_targets, ka, kb, scale


def assemble_output(results, perms, fix_targets, ka, kb, scale):
    parts = []
    for c in range(N_CORES):
        raw = results[c]["out"].reshape(P, (ka + kb) // P, DIM)
        slots = raw.transpose(1, 0, 2).reshape(-1, DIM)  # slot-major
        part = slots[perms[c]].astype(np.float32) * scale
        targets = fix_targets[c]
        if targets:
            fo = results[c]["fixout"]
            for slot, loc in targets.items():
                part[loc] = fo[slot].astype(np.float32) * scale
        parts.append(part)
    return np.concatenate(parts, axis=0).reshape(B, S, DIM)


def kernel(x, emb_table):
    from concourse.bass_utils import run_bass_kernel_spmd

    in_maps, perms, fix_targets, ka, kb, scale = shard_inputs(x, emb_table)
    nc = get_nc(ka, kb)
    res = run_bass_kernel_spmd(nc, in_maps, core_ids=list(range(N_CORES)))
    return assemble_output(res.results, perms, fix_targets, ka, kb, scale)


# revision 15
# speedup vs baseline: 1.1091x; 1.1091x over previous
"""Trainium2 Bass kernel for BlankEmbedding (embedding lookup + blank shift-accumulate).

Reference semantics:
    out = emb[x]                                    # [B, S, D] gather
    preblank[p] = (x[p+1]==BLANK) & (x[p]!=BLANK)   (per row; zero-padded shifts)
    out[p+k] += preblank[p] * emb[x[p]]  for k in 1..3

Strategy: data-parallel over the 16384 flattened tokens, 2048 per core.
The device gathers int8-quantized rows (global absmax/127 scale; ~7.8e-3
rel err vs the 2e-2 budget) and stores them unmodified; the host applies
the scale while unsharding. Sparse blank fixups (P(blank)=1/50257) are
recomputed on-device in int16 and placed by the host.

- Gathers run on the SWDGE indirect-DMA path: descgen is the bottleneck
  (~1.1us per 128-row instruction, engine-serial; measured that neither
  multiple SWDGE queues nor InstDMAGatherAnt beat it once its ~9us mlp
  ucode library load is accounted). Layout ix[p, j] = token 16p + j, so
  each partition holds 16 consecutive tokens and each store descriptor
  is contiguous in DRAM.
- int8 end-to-end halves both the random-row reads (1KB rows) and the
  store traffic vs the bf16 variant, and removes the DVE dequant stage.
- The two fixup gathers sit right after the first main gather so their
  adds + fixout store complete under the main chain instead of tailing
  it. Unused fixup slots read the appended zero row (index VOCAB).
"""

import numpy as np

VOCAB = 50257
ZROW = VOCAB                 # appended all-zeros table row (no-op addend)
DIM = 1024
BLANK = 100
N_BLANKS = 3
B, S = 4, 4096
N_CORES = 8
TOK = B * S                  # 16384 flattened tokens
TPC = TOK // N_CORES         # 2048 tokens per core
P = 128                      # SBUF partitions
NJ = TPC // P                # 16 tokens per partition

_CACHE = {}


def _build_nc(kfix=16, has2=False):
    from concourse import bacc, mybir, tile
    import concourse.bass as bass

    nc = bacc.Bacc(
        "TRN2", target_bir_lowering=False, debug=False, num_devices=1
    )
    i8 = mybir.dt.int8
    i16 = mybir.dt.int16
    i32 = mybir.dt.int32

    ix_dram = nc.dram_tensor("ix", [P, NJ], i32, kind="ExternalInput")
    emb8 = nc.dram_tensor("emb8", [VOCAB + 1, DIM], i8, kind="ExternalInput")
    fix_dram = nc.dram_tensor("fix", [P, 3], i32, kind="ExternalInput")
    out = nc.dram_tensor("out", [TPC, DIM], i8, kind="ExternalOutput")
    fixout = nc.dram_tensor("fixout", [kfix, DIM], i16, kind="ExternalOutput")

    with tile.TileContext(nc) as tc:
        with tc.tile_pool(name="sbuf", bufs=1) as pool:
            ix_all = pool.tile([P, NJ], i32)
            fix_sb = pool.tile([P, 3], i32)  # cols: xt, s1, s2
            nc.sync.dma_start(out=ix_all[:], in_=ix_dram[:])
            nc.scalar.dma_start(out=fix_sb[:], in_=fix_dram[:])

            g8 = pool.tile([P, NJ * DIM], i8)
            out3 = out[:].rearrange("(p j) d -> p j d", p=P, j=NJ)

            def main_gather(j):
                nc.gpsimd.indirect_dma_start(
                    out=g8[:, j * DIM : (j + 1) * DIM],
                    out_offset=None,
                    in_=emb8[:],
                    in_offset=bass.IndirectOffsetOnAxis(
                        ap=ix_all[:, j : j + 1], axis=0
                    ),
                )
                nc.sync.dma_start(
                    out=out3[:, j : j + 1, :],
                    in_=g8[:, j * DIM : (j + 1) * DIM],
                )

            # fixup gathers ride second/third in the descgen chain so the
            # whole fixup path finishes under the main chain's shadow
            main_gather(0)
            ab = pool.tile([P, DIM], i8)
            a1 = pool.tile([P, DIM], i8)
            cols = ((ab, 0), (a1, 1))
            if has2:
                a2 = pool.tile([P, DIM], i8)
                cols += ((a2, 2),)
            for t, col in cols:
                nc.gpsimd.indirect_dma_start(
                    out=t[:kfix, :], out_offset=None, in_=emb8[:],
                    in_offset=bass.IndirectOffsetOnAxis(
                        ap=fix_sb[:kfix, col : col + 1], axis=0
                    ),
                )
            for j in range(1, NJ):
                main_gather(j)

            # fixout[k] = emb8[xt_k] + emb8[s1_k] (+ emb8[s2_k]) in int16
            w0 = pool.tile([P, DIM], i16)
            w1 = pool.tile([P, DIM], i16)
            wide = [w0, w1]
            if has2:
                w2 = pool.tile([P, DIM], i16)
                wide.append(w2)
            for (t, _), w in zip(cols, wide):
                nc.vector.tensor_scalar(
                    out=w[:kfix, :], in0=t[:kfix, :],
                    scalar1=1.0, scalar2=None, op0=mybir.AluOpType.mult,
                )
            if has2:
                nc.vector.tensor_tensor(
                    out=w1[:kfix, :], in0=w1[:kfix, :], in1=w2[:kfix, :],
                    op=mybir.AluOpType.add,
                )
            nc.vector.tensor_tensor(
                out=w0[:kfix, :], in0=w0[:kfix, :], in1=w1[:kfix, :],
                op=mybir.AluOpType.add,
            )
            nc.scalar.dma_start(out=fixout[:], in_=w0[:kfix, :])

    nc.compile()
    return nc


def get_nc(kfix=16, has2=False):
    key = (kfix, has2)
    if key not in _CACHE:
        _CACHE[key] = _build_nc(kfix, has2)
    return _CACHE[key]


def _corrections(x2):
    """Exact reference semantics: list of (global_target_row, src_token)."""
    is_blank = x2 == BLANK
    prev = np.zeros_like(is_blank)
    prev[:, 1:] = is_blank[:, :-1]
    first_blank = is_blank & ~prev
    out = []
    for b, f in np.argwhere(first_blank):
        if f == 0:
            continue  # run at row start: reference shifts in zeros
        p = f - 1
        src_tok = int(x2[b, p])
        for k in range(1, N_BLANKS + 1):
            s = p + k
            if s >= S:
                break
            out.append((b * S + s, src_tok))
    return out


def shard_inputs(x, emb_table):
    """Returns (in_maps, fix_targets, kfix, has2, scale); fix_targets[c]
    maps fixout slot -> core-local target row."""
    x2 = np.asarray(x).astype(np.int64).reshape(B, S)
    flat = x2.reshape(-1).astype(np.int32)
    emb_f = np.asarray(emb_table, dtype=np.float32)
    scale = float(np.abs(emb_f).max()) / 127.0
    emb_i8 = np.vstack(
        [
            np.clip(np.rint(emb_f / scale), -127, 127).astype(np.int8),
            np.zeros((1, DIM), dtype=np.int8),
        ]
    )

    # per-target slots: tgt -> up to 2 src tokens (two blank runs can land
    # on one target only at distance 2; adjacent first-blanks are impossible)
    per_tgt = {}
    for tgt, src in _corrections(x2):
        per_tgt.setdefault(tgt, []).append(src)
    assert all(len(v) <= 2 for v in per_tgt.values()), per_tgt
    has2 = any(len(v) > 1 for v in per_tgt.values())
    max_per_core = max(
        sum(1 for t in per_tgt if c * TPC <= t < (c + 1) * TPC)
        for c in range(N_CORES)
    )
    kfix = 16 if max_per_core <= 16 else P

    in_maps = []
    fix_targets = []
    for c in range(N_CORES):
        base = c * TPC
        ix = np.ascontiguousarray(flat[base : base + TPC].reshape(P, NJ))

        fix = np.full((P, 3), ZROW, dtype=np.int32)  # xt, s1, s2
        fix[:, 0] = 0  # unused slots recompute emb[0]+0+0; host ignores them
        mine = {t: v for t, v in per_tgt.items() if base <= t < base + TPC}
        assert len(mine) <= kfix, "fixup slot overflow"
        targets = {}
        for slot, (t, srcs) in enumerate(mine.items()):
            fix[slot] = [flat[t], srcs[0], srcs[1] if len(srcs) > 1 else ZROW]
            targets[slot] = t - base
        fix_targets.append(targets)
        in_maps.append({"ix": ix, "emb8": emb_i8, "fix": fix})
    return in_maps, fix_targets, kfix, has2, scale


def assemble_output(results, fix_targets, scale):
    parts = []
    for c in range(N_CORES):
        part = results[c]["out"].astype(np.float32) * scale
        targets = fix_targets[c]
        if targets:
            fo = results[c]["fixout"]
            for slot, loc in targets.items():
                part[loc] = fo[slot].astype(np.float32) * scale
        parts.append(part)
    return np.concatenate(parts, axis=0).reshape(B, S, DIM)


def kernel(x, emb_table):
    from concourse.bass_utils import run_bass_kernel_spmd

    in_maps, fix_targets, kfix, has2, scale = shard_inputs(x, emb_table)
    nc = get_nc(kfix, has2)
    res = run_bass_kernel_spmd(nc, in_maps, core_ids=list(range(N_CORES)))
    return assemble_output(res.results, fix_targets, scale)
